# revision 30
# baseline (speedup 1.0000x reference)
"""AggregateAttention kernel for 8x TRN2 NeuronCores (Bass/Tile).

Reference computation (B=4, A=2, R=36, D=2048, N=500):
    x_wx  = x[b,r] @ wx[a,r] + wx_bias[a,r]          # (b,a,r,1,d')
    y_wy  = pool[r] @ wy[r] + wy_bias[r]             # (r,n,d)
    M     = softmax_n(x_wx @ y_wy^T / sqrt(D))       # (b,a,r,1,n)
    out   = M @ pool[r]                              # (b,a,r,1,d)

Key restructuring:
  * scores = x_wx @ wy^T @ pool^T / sqrt(D).  Associating left-to-right
    ((x_wx @ wy^T) @ pool^T) keeps every intermediate at 8 rows per r,
    avoiding the 151-GFLOP pool@wy product entirely (~6 GFLOP total).
  * wy_bias adds a constant per score row -> drops out under softmax.
  * 1/sqrt(D) is folded into x^T and wx_bias on the host.
  * all weight/pool streams are bf16 (harness tolerance 2e-2, measured
    rel err 8.3e-3): halves HBM traffic AND doubles PE matmul rate vs
    fp32r, taking the kernel from HBM-bound to the ridge.

Sharding (uniform SPMD structure, one compiled program):
  72 (a,r) units over 8 cores = 9 units/core, grouped into 5 r-slots of
  [2,2,2,2,1] units.  Core c owns whole r in {4c..4c+3} (both a) plus one
  half region (r = 32 + c//2, a = c%2).  Every core moves ~138 MB of HBM
  (bf16), within ~15% of the compulsory-traffic roofline at 358 GB/s.

Buffer tuning (perfetto-driven; SBUF is ~207 KB/partition and FULL):
  * each slot is DMA-bound overall (~88 us of traffic vs ~68 us of PE),
    but the PE-heavy P2..P5 stretch drains no wx, so the wx prefetch
    depth decides whether the 16 DMA queues idle mid-slot.  wxp=5 (10 MB
    ahead) + half-size pool tiles (ptw=8/pnw=2, freeing 31 KB) removed
    most of two ~10 us all-queues-idle windows per slot (487->458 us).
  * measured dead ends: DMA-XBAR transposes (256B descriptors, +90 us),
    fp8 anything (4-5% output error), on-chip poolT via 64 PE transposes
    per slot (+10 us/slot PE doesn't fit phase-1's scattered idle gaps,
    549 us), per-unit copy-split across Scalar/DVE with SBUF->SBUF merge
    (intermittent NaN race).
  * still open: software-pipelining P1(s+1) into slot s's PE stretch
    (needs ps_o split into [8,1024] PSUM halves to coexist with ps_x),
    and the ~24 us final-slot tail.  Host env jitter is +/-6%; validate
    any change with >=3 runs.
"""

import math
import os

import numpy as np

B, A, R, D, NPOOL = 4, 2, 36, 2048, 500
NCORES = 8
KC = D // 128            # 16 contraction chunks of 128
FD = 512                 # matmul moving-operand free-dim chunk (fp32 PSUM bank)
NF = D // FD             # 4 free-dim chunks
NCH = [128, 128, 128, NPOOL - 3 * 128]   # pool n-chunks (last = 116)
SLOT_UNITS = [2, 2, 2, 2, 1]             # units per r-slot (uniform across cores)
CW = 2                   # contraction chunks per wx/wyT dma_start (2 MB fp32)
PTW = 8                  # poolT chunks per dma_start
PNW = 2                  # pool n-chunks per dma_start

# matmul dtype mode: "f32r" (fp32 data, fp32r matmul), "f32", or "bf16"
MM_MODE = os.environ.get("KERNEL_MM_MODE", "bf16")

# stash for test harness introspection (ignored by grading)
LAST_RESULTS = None


def _stream_cfg(mode):
    """Per-stream storage dtype: 'b' = bf16, 'r' = fp32 data via fp32r."""
    if mode == "bf16":
        return {"wx": "b", "wy": "b", "pt": "b", "pn": "b"}
    if mode == "hyb_wx":
        return {"wx": "b", "wy": "r", "pt": "r", "pn": "r"}
    if mode == "hyb_wxy":
        return {"wx": "b", "wy": "b", "pt": "r", "pn": "r"}
    if mode == "hyb_pool32":
        return {"wx": "b", "wy": "b", "pt": "b", "pn": "r"}
    return {"wx": "r", "wy": "r", "pt": "r", "pn": "r"}


def _assignment(c):
    """Return (units, rslots) for core c.

    units: list of 9 (r, a) pairs in slot order.
    rslots: the 5 distinct r values, one per slot.
    """
    units = []
    rslots = []
    for s in range(4):
        r = 4 * c + s
        units += [(r, 0), (r, 1)]
        rslots.append(r)
    rh, ah = 32 + c // 2, c % 2
    units.append((rh, ah))
    rslots.append(rh)
    return units, rslots


def build_nc(mode=MM_MODE):
    import concourse.bass as bass
    import concourse.bacc as bacc
    import concourse.mybir as mybir
    import concourse.tile as tile
    from contextlib import ExitStack
    from concourse.masks import make_identity

    f32 = mybir.dt.float32
    DTMAP = {"b": mybir.dt.bfloat16, "r": mybir.dt.float32r, "f": f32}
    # walrus requires fp32r matmul operands to be *typed* fp32r all the way
    # from their producer (DMA or rounding engine write), hence "r" tiles.
    cfg = _stream_cfg(mode)
    st_wx = DTMAP[cfg["wx"]]
    st_wy = DTMAP[cfg["wy"]]
    st_pt = DTMAP[cfg["pt"]]
    st_pn = DTMAP[cfg["pn"]]

    # Bacc (not raw Bass): its compile() runs move_matmul_waits_to_ldweights
    # + generate_event_semaphores, which split excess per-instruction sem
    # waits (TRN2 caps most instructions at ONE wait).
    nc = bacc.Bacc(trn_type="TRN2")

    # all big tensors are host-packed partition-major: [.., p=128, chunk, free]
    # so one dma_start moves CW chunks with contiguous per-partition lines
    wx_d = nc.declare_dram_parameter("wx_sh", [9, 128, KC, D], st_wx, isOutput=False)
    wxb_d = nc.declare_dram_parameter("wxb_sh", [9, D], st_wx, isOutput=False)
    xT_d = nc.declare_dram_parameter("xT_sh", [128, 9, KC, B], st_wx, isOutput=False)
    wyT_d = nc.declare_dram_parameter("wyT_sh", [5, 128, KC, D], st_wy, isOutput=False)
    poolT_d = nc.declare_dram_parameter("poolT_sh", [5, 128, KC, NPOOL], st_pt, isOutput=False)
    pool_d = nc.declare_dram_parameter("pool_sh", [5, 128, len(NCH), D], st_pn, isOutput=False)
    ones_d = nc.declare_dram_parameter("ones_sh", [1, B], st_wx, isOutput=False)
    out_d = nc.declare_dram_parameter("out", [4 * 9, D], f32, isOutput=True)

    def mv(ap):
        return ap

    Exp = mybir.ActivationFunctionType.Exp
    Copy = mybir.ActivationFunctionType.Copy
    AX = mybir.AxisListType.X

    # chunk counts per dma_start, sized for ~2 MB transfers
    cwx = 4 if cfg["wx"] == "b" else 2
    cwy = 4 if cfg["wy"] == "b" else 2
    ptw = 8
    pnw = 2

    with tile.TileContext(nc) as tc, ExitStack() as ctx:
        singles = ctx.enter_context(tc.tile_pool(name="singles", bufs=1))
        wxp = ctx.enter_context(tc.tile_pool(name="wxp", bufs=5))
        wyp = ctx.enter_context(tc.tile_pool(name="wyp", bufs=3))
        ptp = ctx.enter_context(tc.tile_pool(name="ptp", bufs=2))
        pnp = ctx.enter_context(tc.tile_pool(name="pnp", bufs=2))
        smalls = ctx.enter_context(tc.tile_pool(name="smalls", bufs=1))
        psmm = ctx.enter_context(tc.tile_pool(name="psmm", bufs=1, space="PSUM"))
        pstr = ctx.enter_context(tc.tile_pool(name="pstr", bufs=3, space="PSUM"))
        pss = ctx.enter_context(tc.tile_pool(name="pss", bufs=1, space="PSUM"))

        # constants
        id8 = singles.tile([8, 8], f32)
        make_identity(nc, id8)
        ones = singles.tile([1, B], st_wx)
        nc.sync.dma_start(out=ones, in_=ones_d[:])
        xT_t = singles.tile([128, 9, KC, B], st_wx)
        nc.sync.dma_start(out=xT_t, in_=xT_d[:])

        # TRN2 caps most instructions (incl. the fp32r self-loading matmul)
        # at ONE sync-wait; Bacc splits the excess onto EventSemaphore
        # instructions.  These tiny "absorb" matmuls advance the PE vector
        # clock past cross-engine ticks first (Tile emits a wait only when
        # the engine clock is behind), so the real matmuls only ever wait on
        # their own streaming DMA and no event-sem splitting is needed on
        # the critical path.
        def absorb_mm(ap):
            n = ap.free_size()
            scr = pstr.tile([128, 8], f32, tag="tr")
            nc.tensor.matmul(scr[0:n, 0:n], ap, ap, start=True, stop=True)

        scr0 = pstr.tile([128, 8], f32, tag="tr")
        nc.tensor.transpose(scr0[0:8, 0:8], id8, id8)
        absorb_mm(xT_t[:, 0, 0, :])
        absorb_mm(ones)

        prev_out_sb = None
        urow = 0
        for s in range(5):
            nu = SLOT_UNITS[s]
            P = 4 * nu

            if prev_out_sb is not None:
                # absorb previous slot's ACT tick (out_sb scaled copy)
                absorb_mm(prev_out_sb[0:B, 0:B])

            # ---- Phase 1: x_wx = x @ wx + bias.  Both units accumulate
            # into one [40,D] PSUM tile (unit 1 at partition 32; matmul
            # PSUM bases must be 0/32/64), so unit 1's matmuls don't wait
            # on unit 0's copy-out.
            ps_x = psmm.tile([40, D], f32, tag="mm")
            xwx_sb = []
            for ui in range(nu):
                u = 2 * s + ui
                pb = 32 * ui
                for k0 in range(0, KC, cwx):
                    wxt = wxp.tile([128, cwx, D], st_wx, tag="wx")
                    nc.sync.dma_start(out=wxt, in_=wx_d[u, :, k0:k0 + cwx, :])
                    for kk in range(cwx):
                        for n in range(NF):
                            nc.tensor.matmul(
                                ps_x[pb:pb + 4, FD * n:FD * (n + 1)],
                                xT_t[:, u, k0 + kk, :],
                                wxt[:, kk, FD * n:FD * (n + 1)],
                                start=(k0 + kk == 0), stop=False,
                            )
                wxbt = smalls.tile([1, D], st_wx, tag=f"wxb{ui}")
                nc.sync.dma_start(out=wxbt, in_=wxb_d[u:u + 1, :])
                for n in range(NF):
                    nc.tensor.matmul(
                        ps_x[pb:pb + 4, FD * n:FD * (n + 1)],
                        ones,
                        wxbt[:, FD * n:FD * (n + 1)],
                        start=False, stop=True,
                    )
            for ui in range(nu):
                xs = smalls.tile([4, D], f32, tag=f"xwx{ui}")
                nc.vector.tensor_copy(xs, ps_x[32 * ui:32 * ui + 4])
                xwx_sb.append(xs)

            # transpose x_wx into shared (d'-partition, [u0 b, u1 b]) chunks
            xwxT = smalls.tile([128, KC, P], st_wy, tag="xwxT")
            for ui in range(nu):
                for k in range(KC):
                    trp = pstr.tile([128, 8], f32, tag="tr")
                    nc.tensor.transpose(
                        trp[:, 0:B], xwx_sb[ui][:, 128 * k:128 * (k + 1)],
                        id8[0:B, 0:B],
                    )
                    nc.vector.tensor_copy(
                        xwxT[:, k, B * ui:B * ui + B], trp[:, 0:B]
                    )

            # ---- Phase 2: z = x_wx @ wy^T (contract d'), M=P single matmuls
            absorb_mm(xwxT[:, KC - 1, :])
            ps_z = psmm.tile([P, D], f32, tag="mm")
            for k0 in range(0, KC, cwy):
                wyt = wyp.tile([128, cwy, D], st_wy, tag="wy")
                nc.sync.dma_start(out=wyt, in_=wyT_d[s, :, k0:k0 + cwy, :])
                for kk in range(cwy):
                    for n in range(NF):
                        nc.tensor.matmul(
                            ps_z[:, FD * n:FD * (n + 1)],
                            xwxT[:, k0 + kk, :],
                            wyt[:, kk, FD * n:FD * (n + 1)],
                            start=(k0 + kk == 0), stop=(k0 + kk == KC - 1),
                        )

            z_sb = smalls.tile([P, D], f32, tag="z")
            nc.vector.tensor_copy(z_sb, ps_z)
            zT = smalls.tile([128, KC, P], st_pt, tag="zT")
            for k in range(KC):
                trp = pstr.tile([128, 8], f32, tag="tr")
                nc.tensor.transpose(
                    trp[:, 0:P], z_sb[:, 128 * k:128 * (k + 1)], id8[0:P, 0:P]
                )
                nc.vector.tensor_copy(zT[:, k, :], trp[:, 0:P])

            # ---- Phase 3: scores = z @ pool^T (contract d)
            absorb_mm(zT[:, KC - 1, :])
            ps_s = pss.tile([P, FD], f32, tag="s")
            for j0 in range(0, KC, ptw):
                ptt = ptp.tile([128, ptw, NPOOL], st_pt, tag="pt")
                nc.sync.dma_start(out=ptt, in_=poolT_d[s, :, j0:j0 + ptw, :])
                for jj in range(ptw):
                    j = j0 + jj
                    nc.tensor.matmul(
                        ps_s[:, 0:NPOOL], zT[:, j, :], ptt[:, jj, :],
                        start=(j == 0), stop=(j == KC - 1),
                    )

            # ---- Phase 4: softmax over n (free axis); keep exp unnormalized
            # stage scores through SBUF so ps_s has a single (DVE) reader
            scores_sb = smalls.tile([P, FD], f32, tag="ssb")
            negmax = smalls.tile([P, 1], f32, tag="negmax")
            sumexp = smalls.tile([P, 1], f32, tag="sumexp")
            recip = smalls.tile([P, 1], f32, tag="recip")
            expn = smalls.tile([P, FD], f32, tag="expn")
            nc.vector.tensor_copy(scores_sb[:, 0:NPOOL], ps_s[:, 0:NPOOL])
            nc.vector.reduce_max(
                negmax, scores_sb[:, 0:NPOOL], axis=AX, negate=True
            )
            nc.scalar.activation(
                expn[:, 0:NPOOL], scores_sb[:, 0:NPOOL],
                Exp, bias=negmax, scale=1.0, accum_out=sumexp,
            )
            nc.vector.reciprocal(recip, sumexp)

            # transpose exp(scores) to (n-partition, bu) chunks
            absorb_mm(expn[0:B, 0:B])
            MT = smalls.tile([128, len(NCH), P], st_pn, tag="MT")
            for j, cj in enumerate(NCH):
                trp = pstr.tile([128, 8], f32, tag="tr")
                nc.tensor.transpose(
                    trp[0:cj, 0:P], expn[:, 128 * j:128 * j + cj], id8[0:P, 0:P]
                )
                nc.vector.tensor_copy(MT[0:cj, j, :], trp[0:cj, 0:P])

            # ---- Phase 5: out = exp(scores) @ pool (contract n), scale 1/sum
            cjL = NCH[-1]
            absorb_mm(MT[0:cjL, len(NCH) - 1, :])
            ps_o = psmm.tile([P, D], f32, tag="mm")
            for j0 in range(0, len(NCH), pnw):
                pnt = pnp.tile([128, pnw, D], st_pn, tag="pn")
                nc.sync.dma_start(out=pnt, in_=pool_d[s, :, j0:j0 + pnw, :])
                for jj in range(pnw):
                    j = j0 + jj
                    cj = NCH[j]
                    for n in range(NF):
                        nc.tensor.matmul(
                            ps_o[:, FD * n:FD * (n + 1)],
                            MT[0:cj, j, :],
                            pnt[0:cj, jj, FD * n:FD * (n + 1)],
                            start=(j == 0), stop=(j == len(NCH) - 1),
                        )

            out_sb = smalls.tile([P, D], f32, tag="osb")
            nc.scalar.activation(out_sb, ps_o, Copy, bias=0.0, scale=recip)
            nc.sync.dma_start(out=out_d[urow:urow + P, :], in_=out_sb)
            urow += P

            prev_out_sb = out_sb

    nc.compile()
    return nc


def _np_dt(code):
    if code == "b":
        import ml_dtypes
        return np.dtype(ml_dtypes.bfloat16)
    return np.dtype(np.float32)


def _build_in_maps(x, pool, wx, wxb, wy, mode):
    cfg = _stream_cfg(mode)
    dt_wx, dt_wy = _np_dt(cfg["wx"]), _np_dt(cfg["wy"])
    dt_pt, dt_pn = _np_dt(cfg["pt"]), _np_dt(cfg["pn"])
    scale = np.float32(1.0 / math.sqrt(D))
    in_maps = []
    for c in range(NCORES):
        units, rslots = _assignment(c)
        # partition-major packs: [.., p, chunk, free] so each dma_start reads
        # contiguous per-partition lines
        wx_sh = np.empty((9, 128, KC, D), dt_wx)
        wxb_sh = np.empty((9, D), dt_wx)
        xT_sh = np.empty((128, 9, KC, B), dt_wx)
        for i, (r, a) in enumerate(units):
            # wx[a,r] is (d, d') with d = 128*k + p -> [p, k, d']
            wx_sh[i] = wx[a, r].reshape(KC, 128, D).transpose(1, 0, 2)
            wxb_sh[i] = wxb[a, r, 0] * scale
            # x[b, r, d] -> [p, c, b] with d = 128*c + p, pre-scaled
            xr = (x[:, r, :] * scale).reshape(B, KC, 128)
            xT_sh[:, i] = xr.transpose(2, 1, 0)
        wyT_sh = np.empty((5, 128, KC, D), dt_wy)
        poolT_sh = np.empty((5, 128, KC, NPOOL), dt_pt)
        pool_sh = np.zeros((5, 128, len(NCH), D), dt_pn)
        for i, r in enumerate(rslots):
            # wy[r].T is (d', d) with d' = 128*k + p -> [p, k, d]
            wyT_sh[i] = wy[r].T.reshape(KC, 128, D).transpose(1, 0, 2)
            # pool[r].T is (d, n) with d = 128*j + p -> [p, j, n]
            poolT_sh[i] = pool[r].T.reshape(KC, 128, NPOOL).transpose(1, 0, 2)
            # pool[r] is (n, d) with n = 128*j + p -> [p, j, d], zero-padded
            pr = np.zeros((len(NCH) * 128, D), dt_pn)
            pr[0:NPOOL] = pool[r]
            pool_sh[i] = pr.reshape(len(NCH), 128, D).transpose(1, 0, 2)
        in_maps.append({
            "wx_sh": wx_sh,
            "wxb_sh": wxb_sh,
            "xT_sh": xT_sh,
            "wyT_sh": wyT_sh,
            "poolT_sh": poolT_sh,
            "pool_sh": pool_sh,
            "ones_sh": np.ones((1, B), dt_wx),
        })
    return in_maps


def _gather(results):
    out = np.empty((B, A, R, 1, D), np.float32)
    for c in range(NCORES):
        units, _ = _assignment(c)
        res = np.asarray(results[c]["out"])
        for i, (r, a) in enumerate(units):
            for b in range(B):
                out[b, a, r, 0, :] = res[4 * i + b]
    return out


def kernel(**inputs):
    global LAST_RESULTS
    from concourse.bass_utils import run_bass_kernel_spmd

    x = np.asarray(inputs["top_region_features"], np.float32)
    pool = np.asarray(inputs["normality_pool"], np.float32)
    wx = np.asarray(inputs["wx"], np.float32)
    wxb = np.asarray(inputs["wx_bias"], np.float32)
    wy = np.asarray(inputs["wy"], np.float32)
    # wy_bias is mathematically irrelevant: it shifts every score in a
    # softmax row by the same constant.

    mode = MM_MODE
    in_maps = _build_in_maps(x, pool, wx, wxb, wy, mode)
    nc = build_nc(mode)

    trace = bool(os.environ.get("KERNEL_TRACE"))
    kw = {}
    if trace:
        kw["trace"] = True
        tc_env = os.environ.get("KERNEL_TRACE_CORES")
        if tc_env:
            kw["trace_cores"] = [int(t) for t in tc_env.split(",")]
    res = run_bass_kernel_spmd(nc, in_maps, list(range(NCORES)), **kw)
    LAST_RESULTS = res
    return _gather(res.results)



# revision 31
# speedup vs baseline: 1.1215x; 1.1215x over previous
"""AggregateAttention kernel for 8x TRN2 NeuronCores (Bass/Tile).

Reference computation (B=4, A=2, R=36, D=2048, N=500):
    x_wx  = x[b,r] @ wx[a,r] + wx_bias[a,r]          # (b,a,r,1,d')
    y_wy  = pool[r] @ wy[r] + wy_bias[r]             # (r,n,d)
    M     = softmax_n(x_wx @ y_wy^T / sqrt(D))       # (b,a,r,1,n)
    out   = M @ pool[r]                              # (b,a,r,1,d)

Key restructuring:
  * scores = x_wx @ wy^T @ pool^T / sqrt(D).  Associating left-to-right
    ((x_wx @ wy^T) @ pool^T) keeps every intermediate at 8 rows per r,
    avoiding the 151-GFLOP pool@wy product entirely (~6 GFLOP total).
  * wy_bias adds a constant per score row -> drops out under softmax.
  * 1/sqrt(D) is folded into x^T and wx_bias on the host.
  * all weight/pool streams are bf16 (harness tolerance 2e-2, measured
    rel err 8.3e-3): halves HBM traffic AND doubles PE matmul rate vs
    fp32r, taking the kernel from HBM-bound to the ridge.

Sharding (uniform SPMD structure, one compiled program):
  72 (a,r) units over 8 cores = 9 units/core, grouped into 5 r-slots of
  [2,2,2,2,1] units.  Core c owns whole r in {4c..4c+3} (both a) plus one
  half region (r = 32 + c//2, a = c%2).  Every core moves ~138 MB of HBM
  (bf16), within ~15% of the compulsory-traffic roofline at 358 GB/s.

Buffer tuning (perfetto-driven; SBUF is ~207 KB/partition and FULL):
  * each slot is DMA-bound overall (~88 us of traffic vs ~68 us of PE),
    but the PE-heavy P2..P5 stretch drains no wx, so the wx prefetch
    depth decides whether the 16 DMA queues idle mid-slot.  wxp=5 (10 MB
    ahead) + half-size pool tiles (ptw=8/pnw=2, freeing 31 KB) removed
    most of two ~10 us all-queues-idle windows per slot (487->458 us).
  * measured dead ends: DMA-XBAR transposes (256B descriptors, +90 us),
    fp8 anything (4-5% output error), on-chip poolT via 64 PE transposes
    per slot (+10 us/slot PE doesn't fit phase-1's scattered idle gaps,
    549 us), per-unit copy-split across Scalar/DVE with SBUF->SBUF merge
    (intermittent NaN race).
  * still open: software-pipelining P1(s+1) into slot s's PE stretch
    (needs ps_o split into [8,1024] PSUM halves to coexist with ps_x),
    and the ~24 us final-slot tail.  Host env jitter is +/-6%; validate
    any change with >=3 runs.
"""

import math
import os

import numpy as np

B, A, R, D, NPOOL = 4, 2, 36, 2048, 500
NCORES = 8
KC = D // 128            # 16 contraction chunks of 128
FD = 512                 # matmul moving-operand free-dim chunk (fp32 PSUM bank)
NF = D // FD             # 4 free-dim chunks
NCH = [128, 128, 128, NPOOL - 3 * 128]   # pool n-chunks (last = 116)
SLOT_UNITS = [2, 2, 2, 2, 1]             # units per r-slot (uniform across cores)
CW = 2                   # contraction chunks per wx/wyT dma_start (2 MB fp32)
PTW = 8                  # poolT chunks per dma_start
PNW = 2                  # pool n-chunks per dma_start

# matmul dtype mode: "f32r" (fp32 data, fp32r matmul), "f32", or "bf16"
MM_MODE = os.environ.get("KERNEL_MM_MODE", "bf16")

# stash for test harness introspection (ignored by grading)
LAST_RESULTS = None


def _stream_cfg(mode):
    """Per-stream storage dtype: 'b' = bf16, 'r' = fp32 data via fp32r."""
    if mode == "bf16":
        return {"wx": "b", "wy": "b", "pt": "b", "pn": "b"}
    if mode == "hyb_wx":
        return {"wx": "b", "wy": "r", "pt": "r", "pn": "r"}
    if mode == "hyb_wxy":
        return {"wx": "b", "wy": "b", "pt": "r", "pn": "r"}
    if mode == "hyb_pool32":
        return {"wx": "b", "wy": "b", "pt": "b", "pn": "r"}
    return {"wx": "r", "wy": "r", "pt": "r", "pn": "r"}


def _assignment(c):
    """Return (units, rslots) for core c.

    units: list of 9 (r, a) pairs in slot order.
    rslots: the 5 distinct r values, one per slot.
    """
    units = []
    rslots = []
    for s in range(4):
        r = 4 * c + s
        units += [(r, 0), (r, 1)]
        rslots.append(r)
    rh, ah = 32 + c // 2, c % 2
    units.append((rh, ah))
    rslots.append(rh)
    return units, rslots


def build_nc(mode=MM_MODE):
    import concourse.bass as bass
    import concourse.bacc as bacc
    import concourse.mybir as mybir
    import concourse.tile as tile
    from contextlib import ExitStack
    from concourse.masks import make_identity

    f32 = mybir.dt.float32
    DTMAP = {"b": mybir.dt.bfloat16, "r": mybir.dt.float32r, "f": f32}
    # walrus requires fp32r matmul operands to be *typed* fp32r all the way
    # from their producer (DMA or rounding engine write), hence "r" tiles.
    cfg = _stream_cfg(mode)
    st_wx = DTMAP[cfg["wx"]]
    st_wy = DTMAP[cfg["wy"]]
    st_pt = DTMAP[cfg["pt"]]
    st_pn = DTMAP[cfg["pn"]]

    # Bacc (not raw Bass): its compile() runs move_matmul_waits_to_ldweights
    # + generate_event_semaphores, which split excess per-instruction sem
    # waits (TRN2 caps most instructions at ONE wait).
    nc = bacc.Bacc(trn_type="TRN2")

    # all big tensors are host-packed partition-major: [.., p=128, chunk, free]
    # so one dma_start moves CW chunks with contiguous per-partition lines
    wx_d = nc.declare_dram_parameter("wx_sh", [9, 128, KC, D], st_wx, isOutput=False)
    wxb_d = nc.declare_dram_parameter("wxb_sh", [9, D], st_wx, isOutput=False)
    xT_d = nc.declare_dram_parameter("xT_sh", [128, 9, KC, B], st_wx, isOutput=False)
    wyT_d = nc.declare_dram_parameter("wyT_sh", [5, 128, KC, D], st_wy, isOutput=False)
    poolT_d = nc.declare_dram_parameter("poolT_sh", [5, 128, KC, NPOOL], st_pt, isOutput=False)
    pool_d = nc.declare_dram_parameter("pool_sh", [5, 128, len(NCH), D], st_pn, isOutput=False)
    ones_d = nc.declare_dram_parameter("ones_sh", [1, B], st_wx, isOutput=False)
    out_d = nc.declare_dram_parameter("out", [4 * 9, D], f32, isOutput=True)

    def mv(ap):
        return ap

    Exp = mybir.ActivationFunctionType.Exp
    Copy = mybir.ActivationFunctionType.Copy
    AX = mybir.AxisListType.X

    # chunk counts per dma_start, sized for ~2 MB transfers
    cwx = 4 if cfg["wx"] == "b" else 2
    cwy = 4 if cfg["wy"] == "b" else 2
    ptw = 8
    pnw = 2

    with tile.TileContext(nc) as tc, ExitStack() as ctx:
        singles = ctx.enter_context(tc.tile_pool(name="singles", bufs=1))
        wxp = ctx.enter_context(tc.tile_pool(name="wxp", bufs=6))
        wyp = ctx.enter_context(tc.tile_pool(name="wyp", bufs=2))
        ptp = ctx.enter_context(tc.tile_pool(name="ptp", bufs=2))
        pnp = ctx.enter_context(tc.tile_pool(name="pnp", bufs=2))
        smalls = ctx.enter_context(tc.tile_pool(name="smalls", bufs=1))
        psmm = ctx.enter_context(tc.tile_pool(name="psmm", bufs=1, space="PSUM"))
        pstr = ctx.enter_context(tc.tile_pool(name="pstr", bufs=3, space="PSUM"))
        pss = ctx.enter_context(tc.tile_pool(name="pss", bufs=1, space="PSUM"))

        # constants
        id8 = singles.tile([8, 8], f32)
        make_identity(nc, id8)
        ones = singles.tile([1, B], st_wx)
        nc.sync.dma_start(out=ones, in_=ones_d[:])
        xT_t = singles.tile([128, 9, KC, B], st_wx)
        nc.sync.dma_start(out=xT_t, in_=xT_d[:])

        # TRN2 caps most instructions (incl. the fp32r self-loading matmul)
        # at ONE sync-wait; Bacc splits the excess onto EventSemaphore
        # instructions.  These tiny "absorb" matmuls advance the PE vector
        # clock past cross-engine ticks first (Tile emits a wait only when
        # the engine clock is behind), so the real matmuls only ever wait on
        # their own streaming DMA and no event-sem splitting is needed on
        # the critical path.
        def absorb_mm(ap):
            n = ap.free_size()
            scr = pstr.tile([128, 8], f32, tag="tr")
            nc.tensor.matmul(scr[0:n, 0:n], ap, ap, start=True, stop=True)

        scr0 = pstr.tile([128, 8], f32, tag="tr")
        nc.tensor.transpose(scr0[0:8, 0:8], id8, id8)
        absorb_mm(xT_t[:, 0, 0, :])
        absorb_mm(ones)

        prev_out_sb = None
        urow = 0
        for s in range(5):
            nu = SLOT_UNITS[s]
            P = 4 * nu

            if prev_out_sb is not None:
                # absorb previous slot's ACT tick (out_sb scaled copy)
                absorb_mm(prev_out_sb[0:B, 0:B])

            # ---- Phase 1: x_wx = x @ wx + bias.  Both units accumulate
            # into one [40,D] PSUM tile (unit 1 at partition 32; matmul
            # PSUM bases must be 0/32/64), so unit 1's matmuls don't wait
            # on unit 0's copy-out.
            ps_x = psmm.tile([40, D], f32, tag="mm")
            xwx_sb = []
            for ui in range(nu):
                u = 2 * s + ui
                pb = 32 * ui
                for k0 in range(0, KC, cwx):
                    wxt = wxp.tile([128, cwx, D], st_wx, tag="wx")
                    nc.sync.dma_start(out=wxt, in_=wx_d[u, :, k0:k0 + cwx, :])
                    for kk in range(cwx):
                        for n in range(NF):
                            nc.tensor.matmul(
                                ps_x[pb:pb + 4, FD * n:FD * (n + 1)],
                                xT_t[:, u, k0 + kk, :],
                                wxt[:, kk, FD * n:FD * (n + 1)],
                                start=(k0 + kk == 0), stop=False,
                            )
                wxbt = smalls.tile([1, D], st_wx, tag=f"wxb{ui}")
                nc.sync.dma_start(out=wxbt, in_=wxb_d[u:u + 1, :])
                for n in range(NF):
                    nc.tensor.matmul(
                        ps_x[pb:pb + 4, FD * n:FD * (n + 1)],
                        ones,
                        wxbt[:, FD * n:FD * (n + 1)],
                        start=False, stop=True,
                    )
            for ui in range(nu):
                xs = smalls.tile([4, D], f32, tag=f"xwx{ui}")
                nc.vector.tensor_copy(xs, ps_x[32 * ui:32 * ui + 4])
                xwx_sb.append(xs)

            # transpose x_wx into shared (d'-partition, [u0 b, u1 b]) chunks
            xwxT = smalls.tile([128, KC, P], st_wy, tag="xwxT")
            for ui in range(nu):
                for k in range(KC):
                    trp = pstr.tile([128, 8], f32, tag="tr")
                    nc.tensor.transpose(
                        trp[:, 0:B], xwx_sb[ui][:, 128 * k:128 * (k + 1)],
                        id8[0:B, 0:B],
                    )
                    nc.vector.tensor_copy(
                        xwxT[:, k, B * ui:B * ui + B], trp[:, 0:B]
                    )

            # ---- Phase 2: z = x_wx @ wy^T (contract d'), M=P single matmuls
            absorb_mm(xwxT[:, KC - 1, :])
            ps_z = psmm.tile([P, D], f32, tag="mm")
            for k0 in range(0, KC, cwy):
                wyt = wyp.tile([128, cwy, D], st_wy, tag="wy")
                nc.sync.dma_start(out=wyt, in_=wyT_d[s, :, k0:k0 + cwy, :])
                for kk in range(cwy):
                    for n in range(NF):
                        nc.tensor.matmul(
                            ps_z[:, FD * n:FD * (n + 1)],
                            xwxT[:, k0 + kk, :],
                            wyt[:, kk, FD * n:FD * (n + 1)],
                            start=(k0 + kk == 0), stop=(k0 + kk == KC - 1),
                        )

            z_sb = smalls.tile([P, D], f32, tag="z")
            nc.vector.tensor_copy(z_sb, ps_z)
            zT = smalls.tile([128, KC, P], st_pt, tag="zT")
            for k in range(KC):
                trp = pstr.tile([128, 8], f32, tag="tr")
                nc.tensor.transpose(
                    trp[:, 0:P], z_sb[:, 128 * k:128 * (k + 1)], id8[0:P, 0:P]
                )
                nc.vector.tensor_copy(zT[:, k, :], trp[:, 0:P])

            # ---- Phase 3: scores = z @ pool^T (contract d)
            absorb_mm(zT[:, KC - 1, :])
            ps_s = pss.tile([P, FD], f32, tag="s")
            for j0 in range(0, KC, ptw):
                ptt = ptp.tile([128, ptw, NPOOL], st_pt, tag="pt")
                nc.sync.dma_start(out=ptt, in_=poolT_d[s, :, j0:j0 + ptw, :])
                for jj in range(ptw):
                    j = j0 + jj
                    nc.tensor.matmul(
                        ps_s[:, 0:NPOOL], zT[:, j, :], ptt[:, jj, :],
                        start=(j == 0), stop=(j == KC - 1),
                    )

            # ---- Phase 4: softmax over n (free axis); keep exp unnormalized
            # stage scores through SBUF so ps_s has a single (DVE) reader
            scores_sb = smalls.tile([P, FD], f32, tag="ssb")
            negmax = smalls.tile([P, 1], f32, tag="negmax")
            sumexp = smalls.tile([P, 1], f32, tag="sumexp")
            recip = smalls.tile([P, 1], f32, tag="recip")
            expn = smalls.tile([P, FD], f32, tag="expn")
            nc.vector.tensor_copy(scores_sb[:, 0:NPOOL], ps_s[:, 0:NPOOL])
            nc.vector.reduce_max(
                negmax, scores_sb[:, 0:NPOOL], axis=AX, negate=True
            )
            nc.scalar.activation(
                expn[:, 0:NPOOL], scores_sb[:, 0:NPOOL],
                Exp, bias=negmax, scale=1.0, accum_out=sumexp,
            )
            nc.vector.reciprocal(recip, sumexp)

            # transpose exp(scores) to (n-partition, bu) chunks
            absorb_mm(expn[0:B, 0:B])
            MT = smalls.tile([128, len(NCH), P], st_pn, tag="MT")
            for j, cj in enumerate(NCH):
                trp = pstr.tile([128, 8], f32, tag="tr")
                nc.tensor.transpose(
                    trp[0:cj, 0:P], expn[:, 128 * j:128 * j + cj], id8[0:P, 0:P]
                )
                nc.vector.tensor_copy(MT[0:cj, j, :], trp[0:cj, 0:P])

            # ---- Phase 5: out = exp(scores) @ pool (contract n), scale 1/sum
            cjL = NCH[-1]
            absorb_mm(MT[0:cjL, len(NCH) - 1, :])
            ps_o = psmm.tile([P, D], f32, tag="mm")
            for j0 in range(0, len(NCH), pnw):
                pnt = pnp.tile([128, pnw, D], st_pn, tag="pn")
                nc.sync.dma_start(out=pnt, in_=pool_d[s, :, j0:j0 + pnw, :])
                for jj in range(pnw):
                    j = j0 + jj
                    cj = NCH[j]
                    for n in range(NF):
                        nc.tensor.matmul(
                            ps_o[:, FD * n:FD * (n + 1)],
                            MT[0:cj, j, :],
                            pnt[0:cj, jj, FD * n:FD * (n + 1)],
                            start=(j == 0), stop=(j == len(NCH) - 1),
                        )

            out_sb = smalls.tile([P, D], f32, tag="osb")
            nc.scalar.activation(out_sb, ps_o, Copy, bias=0.0, scale=recip)
            nc.sync.dma_start(out=out_d[urow:urow + P, :], in_=out_sb)
            urow += P

            prev_out_sb = out_sb

    nc.compile()
    return nc


def _np_dt(code):
    if code == "b":
        import ml_dtypes
        return np.dtype(ml_dtypes.bfloat16)
    return np.dtype(np.float32)


def _build_in_maps(x, pool, wx, wxb, wy, mode):
    cfg = _stream_cfg(mode)
    dt_wx, dt_wy = _np_dt(cfg["wx"]), _np_dt(cfg["wy"])
    dt_pt, dt_pn = _np_dt(cfg["pt"]), _np_dt(cfg["pn"])
    scale = np.float32(1.0 / math.sqrt(D))
    in_maps = []
    for c in range(NCORES):
        units, rslots = _assignment(c)
        # partition-major packs: [.., p, chunk, free] so each dma_start reads
        # contiguous per-partition lines
        wx_sh = np.empty((9, 128, KC, D), dt_wx)
        wxb_sh = np.empty((9, D), dt_wx)
        xT_sh = np.empty((128, 9, KC, B), dt_wx)
        for i, (r, a) in enumerate(units):
            # wx[a,r] is (d, d') with d = 128*k + p -> [p, k, d']
            wx_sh[i] = wx[a, r].reshape(KC, 128, D).transpose(1, 0, 2)
            wxb_sh[i] = wxb[a, r, 0] * scale
            # x[b, r, d] -> [p, c, b] with d = 128*c + p, pre-scaled
            xr = (x[:, r, :] * scale).reshape(B, KC, 128)
            xT_sh[:, i] = xr.transpose(2, 1, 0)
        wyT_sh = np.empty((5, 128, KC, D), dt_wy)
        poolT_sh = np.empty((5, 128, KC, NPOOL), dt_pt)
        pool_sh = np.zeros((5, 128, len(NCH), D), dt_pn)
        for i, r in enumerate(rslots):
            # wy[r].T is (d', d) with d' = 128*k + p -> [p, k, d]
            wyT_sh[i] = wy[r].T.reshape(KC, 128, D).transpose(1, 0, 2)
            # pool[r].T is (d, n) with d = 128*j + p -> [p, j, n]
            poolT_sh[i] = pool[r].T.reshape(KC, 128, NPOOL).transpose(1, 0, 2)
            # pool[r] is (n, d) with n = 128*j + p -> [p, j, d], zero-padded
            pr = np.zeros((len(NCH) * 128, D), dt_pn)
            pr[0:NPOOL] = pool[r]
            pool_sh[i] = pr.reshape(len(NCH), 128, D).transpose(1, 0, 2)
        in_maps.append({
            "wx_sh": wx_sh,
            "wxb_sh": wxb_sh,
            "xT_sh": xT_sh,
            "wyT_sh": wyT_sh,
            "poolT_sh": poolT_sh,
            "pool_sh": pool_sh,
            "ones_sh": np.ones((1, B), dt_wx),
        })
    return in_maps


def _gather(results):
    out = np.empty((B, A, R, 1, D), np.float32)
    for c in range(NCORES):
        units, _ = _assignment(c)
        res = np.asarray(results[c]["out"])
        for i, (r, a) in enumerate(units):
            for b in range(B):
                out[b, a, r, 0, :] = res[4 * i + b]
    return out


def kernel(**inputs):
    global LAST_RESULTS
    from concourse.bass_utils import run_bass_kernel_spmd

    x = np.asarray(inputs["top_region_features"], np.float32)
    pool = np.asarray(inputs["normality_pool"], np.float32)
    wx = np.asarray(inputs["wx"], np.float32)
    wxb = np.asarray(inputs["wx_bias"], np.float32)
    wy = np.asarray(inputs["wy"], np.float32)
    # wy_bias is mathematically irrelevant: it shifts every score in a
    # softmax row by the same constant.

    mode = MM_MODE
    in_maps = _build_in_maps(x, pool, wx, wxb, wy, mode)
    nc = build_nc(mode)

    trace = bool(os.environ.get("KERNEL_TRACE"))
    kw = {}
    if trace:
        kw["trace"] = True
        tc_env = os.environ.get("KERNEL_TRACE_CORES")
        if tc_env:
            kw["trace_cores"] = [int(t) for t in tc_env.split(",")]
    res = run_bass_kernel_spmd(nc, in_maps, list(range(NCORES)), **kw)
    LAST_RESULTS = res
    return _gather(res.results)

